# revision 22
# baseline (speedup 1.0000x reference)
"""MultiHeadAttention TRN2 Bass kernel, sharded over 8 NeuronCores.

Sharding: 8 cores = 2 batches x 4 head-groups. Each core computes 4 heads of
one batch end-to-end (q/k/v projections, biased+masked softmax attention, and
a partial output projection); the host sums the per-group partial outputs.

All math is bf16 (fp8 was measured to cost ~4-6% output error here: attention
outputs are means over ~150 effective keys, so softmax averaging shrinks the
signal exactly as much as the quantization noise -- per-element quantization
error passes through at full strength).

Layout/scheduling highlights:
  - fully "transposed" on-device layout: host supplies x^T [D, S] per batch
    (bf16) and per-core weight slices; projections produce qT/kT [dims, S];
    v stays natural [S, dh+1] per chunk.
  - scores are row-tiled: K=64 per head, so the head pair runs CONCURRENTLY
    in two 64x128 PE array tiles (tile_position inferred from the partition
    offsets of the kT/qT halves). No zero-padding.
  - the exp stream on ScalarE is the attention-phase bottleneck; the first
    RAMP score chunks are emitted between the early and late v-projection
    matmuls so ScalarE saturates while PE still runs projections.
  - softmax: exp on ScalarE (PSUM->SBUF bf16), bias/mask applied as one DVE
    multiply with host-precomputed exp(bias)^T; the denominator comes free
    as a ones-column in the attn@v matmul.
  - epilogue (normalize-by-sum) is 3-stage software-pipelined into the next
    pair instance; stage 1 copies the whole out2 PSUM tile to SBUF, which
    releases the PSUM buffer immediately so the next pair never stalls.
  - output projection emits bf16 partial^T [D, S] per core in the tail.
"""

import numpy as np
import ml_dtypes

import concourse.bass as bass
import concourse.mybir as mybir
import concourse.tile as tile
from concourse.bacc import Bacc

BF16 = mybir.dt.bfloat16
F32 = mybir.dt.float32
nbf16 = ml_dtypes.bfloat16

B = 2
S_FULL = 2048
D = 1024
H = 16
DH = 64
HPC = 4  # heads per core
CD = HPC * DH  # 256 per-core projected dims
NCORES = 8
SCALE = 8.0  # sqrt(DH)

KC = D // 128  # 8 contraction chunks for projections
NB = 512  # token-block (free dim per matmul)
ROW_TILE = True  # 64x128 row-tiled scores (head pair concurrent in the array)


def build_module(S=S_FULL):
    """Build the single-core Bass program (same program runs SPMD on 8 cores)."""
    assert S % 1024 == 0
    SUPS = 2  # s_q superblocks
    SUPLEN = S // SUPS  # columns per superblock
    NT = S // NB  # projection token blocks
    TC = S // 128  # token / s_k chunks
    RAMP = 5  # score chunks emitted ahead of the late v projection

    nc = Bacc(None)

    xqT = nc.dram_tensor("xqT", [D, S], BF16, kind="ExternalInput")
    xkT = nc.dram_tensor("xkT", [D, S], BF16, kind="ExternalInput")
    xvT = nc.dram_tensor("xvT", [D, S], BF16, kind="ExternalInput")
    wqT = nc.dram_tensor("wqT", [128, KC * CD], BF16, kind="ExternalInput")
    wkT = nc.dram_tensor("wkT", [128, KC * CD], BF16, kind="ExternalInput")
    wvT = nc.dram_tensor("wvT", [128, KC * CD], BF16, kind="ExternalInput")
    woT = nc.dram_tensor("woT", [128, (CD // 128) * D], BF16, kind="ExternalInput")
    bqc = nc.dram_tensor("bqc", [128, 2], F32, kind="ExternalInput")
    bkc = nc.dram_tensor("bkc", [128, 2], F32, kind="ExternalInput")
    bvc = nc.dram_tensor("bvc", [64, HPC], F32, kind="ExternalInput")
    expbT = nc.dram_tensor("expbT", [S, S], BF16, kind="ExternalInput")
    poutT = nc.dram_tensor("poutT", [D, S], BF16, kind="ExternalOutput")

    with tile.TileContext(nc) as tc:
        with (
            tc.tile_pool(name="statics", bufs=1) as statics,
            tc.tile_pool(name="xs", bufs=8) as xs_pool,
            tc.tile_pool(name="xv", bufs=KC) as xv_pool,
            tc.tile_pool(name="expb", bufs=2) as expb_pool,
            tc.tile_pool(name="e", bufs=2) as e_pool,
            tc.tile_pool(name="abf", bufs=11) as abf_pool,
            tc.tile_pool(name="rec", bufs=2) as rec_pool,
            tc.tile_pool(name="spr", bufs=3) as spread_pool,
            tc.tile_pool(name="rb", bufs=2) as rb_pool,
            tc.tile_pool(name="segt", bufs=1) as seg_pool,
            tc.tile_pool(name="oev", bufs=2) as oev_pool,
            tc.tile_pool(name="psc", bufs=2, space="PSUM") as psc,
            tc.tile_pool(name="pacc", bufs=2, space="PSUM") as pacc,
            tc.tile_pool(name="dsc", bufs=6, space="DRAM") as dram_pool,
        ):
            # ---- static tiles ----
            wq_sb = statics.tile([128, KC, CD], BF16, name="wq_sb")
            wk_sb = statics.tile([128, KC, CD], BF16, name="wk_sb")
            wv_sb = statics.tile([128, KC, CD], BF16, name="wv_sb")
            wo_sb = statics.tile([128, CD // 128, D], BF16, name="wo_sb")
            bq_sb = statics.tile([128, 2], F32, name="bq_sb")
            bk_sb = statics.tile([128, 2], F32, name="bk_sb")
            bv_sb = statics.tile([64, HPC], F32, name="bv_sb")
            qT = [statics.tile([128, S], BF16, name=f"qT{m}") for m in range(2)]
            # kT in natural pair layout: head 2m at partitions 0-63, head
            # 2m+1 at 64-127 (matches qT; feeds the row-tiled scores)
            kT = [statics.tile([128, S], BF16, name=f"kT{m}") for m in range(2)]
            if not ROW_TILE:
                kTh = [
                    statics.tile([128, S], BF16, name=f"kTh{h}") for h in range(HPC)
                ]
                for h in range(HPC):
                    nc.gpsimd.memset(kTh[h], 0.0)
            # v natural layout per s_k chunk, with a ones-column (denominator)
            vv = statics.tile([128, HPC, TC, DH + 1], BF16, name="vv")
            cc = [statics.tile([128, S], BF16, name=f"cc{m}") for m in range(2)]

            nc.sync.dma_start(wq_sb, wqT[:, :].rearrange("p (kc m) -> p kc m", kc=KC))
            nc.sync.dma_start(wk_sb, wkT[:, :].rearrange("p (kc m) -> p kc m", kc=KC))
            nc.sync.dma_start(wv_sb, wvT[:, :].rearrange("p (kc m) -> p kc m", kc=KC))
            nc.sync.dma_start(
                wo_sb, woT[:, :].rearrange("p (kc m) -> p kc m", kc=CD // 128)
            )
            nc.sync.dma_start(bq_sb, bqc[:, :])
            nc.sync.dma_start(bk_sb, bkc[:, :])
            nc.sync.dma_start(bv_sb, bvc[:, :])

            nc.gpsimd.memset(vv[:, :, :, DH : DH + 1], 1.0)

            # ---- phase 1: k then q projections (transposed outputs) ----
            for xdram, w_sb, b_sb, dst in (
                (xkT, wk_sb, bk_sb, kT),
                (xqT, wq_sb, bq_sb, qT),
            ):
                for half in range(NT // 2):
                    xts2 = []
                    for kc in range(KC):
                        xt = xs_pool.tile([128, 2 * NB], BF16, name="xt")
                        nc.sync.dma_start(
                            xt,
                            xdram[
                                kc * 128 : (kc + 1) * 128,
                                half * 2 * NB : (half + 1) * 2 * NB,
                            ],
                        )
                        xts2.append(xt)
                    for mt in range(2):
                        for nt2 in range(2):
                            nt = half * 2 + nt2
                            xts = [t[:, nt2 * NB : (nt2 + 1) * NB] for t in xts2]
                            ps = psc.tile([128, NB], F32, name="ps_proj", tag="psc")
                            for kc in range(KC):
                                nc.tensor.matmul(
                                    ps,
                                    lhsT=w_sb[:, kc, mt * 128 : (mt + 1) * 128],
                                    rhs=xts[kc],
                                    start=(kc == 0),
                                    stop=(kc == KC - 1),
                                )
                            # evacuate on DVE (per-partition bias add) to keep
                            # ScalarE free for the exp stream
                            csl = slice(nt * NB, (nt + 1) * NB)
                            if dst is kT and not ROW_TILE:
                                h0, h1 = 2 * mt, 2 * mt + 1
                                nc.vector.tensor_scalar_add(
                                    kTh[h0][0:64, csl],
                                    ps[0:64, :],
                                    scalar1=b_sb[0:64, mt : mt + 1],
                                )
                                nc.vector.tensor_scalar_add(
                                    kTh[h1][64:128, csl],
                                    ps[64:128, :],
                                    scalar1=b_sb[64:128, mt : mt + 1],
                                )
                            else:
                                nc.vector.tensor_scalar_add(
                                    dst[mt][:, csl], ps, scalar1=b_sb[:, mt : mt + 1]
                                )

            # exp(bias)^T masked, superblocks; one DMA per s_k chunk so
            # attention chunk ck starts as soon as ITS slice has landed
            expb_tiles = []
            for sup in range(SUPS):
                t = expb_pool.tile([128, TC, SUPLEN], BF16, name="expb")
                src = expbT[:, sup * SUPLEN : (sup + 1) * SUPLEN].rearrange(
                    "(c p) q -> p c q", p=128
                )
                for ckd in range(TC):
                    nc.sync.dma_start(t[:, ckd, :], src[:, ckd, :])
                expb_tiles.append(t)

            # ---- attention per-chunk emitters ----
            def emit_scores_pair(sup, mt, ck):
                """Scores for the head pair. Row-tiled: h0 runs in the 64x128
                PE array tile at rows 0-63, h1 in rows 64-127, interleaved
                per hf so the two array tiles stream concurrently."""
                scs = [
                    psc.tile([128, SUPLEN], F32, name="sc", tag="psc")
                    for _ in range(2)
                ]
                for hf in range(2):
                    for h01 in range(2):
                        base = 64 * h01
                        if ROW_TILE:
                            nc.tensor.matmul(
                                scs[h01][:, hf * NB : (hf + 1) * NB],
                                lhsT=kT[mt][base : base + 64, ck * 128 : (ck + 1) * 128],
                                rhs=qT[mt][
                                    base : base + 64,
                                    sup * SUPLEN + hf * NB : sup * SUPLEN + (hf + 1) * NB,
                                ],
                                start=True,
                                stop=True,
                            )
                        else:
                            nc.tensor.matmul(
                                scs[h01][:, hf * NB : (hf + 1) * NB],
                                lhsT=kTh[2 * mt + h01][:, ck * 128 : (ck + 1) * 128],
                                rhs=qT[mt][
                                    :,
                                    sup * SUPLEN + hf * NB : sup * SUPLEN + (hf + 1) * NB,
                                ],
                                start=True,
                                stop=True,
                            )
                return scs

            def emit_exp_mul(sup, ck, sc):
                e = e_pool.tile([128, SUPLEN], BF16, name="e")
                nc.scalar.activation(
                    e, sc, func=mybir.ActivationFunctionType.Exp
                )
                abf = abf_pool.tile([128, SUPLEN], BF16, name="abf")
                nc.vector.tensor_mul(abf, e, expb_tiles[sup][:, ck, :])
                return abf

            def emit_attnv(h, ck, out2, abf):
                for hf in range(2):
                    hsl = slice(hf * NB, (hf + 1) * NB)
                    nc.tensor.matmul(
                        out2[:, hsl],
                        lhsT=vv[:, h, ck, :],
                        rhs=abf[:, hsl],
                        start=(ck == 0),
                        stop=(ck == TC - 1),
                    )

            # ---- epilogue (normalize-by-sum), 3-stage software pipelined ----
            def make_epilogue(sup, h, out2):
                qsl = slice(sup * SUPLEN, (sup + 1) * SUPLEN)
                mt = h // 2
                st = {}

                def s1():
                    # copy the WHOLE out2 to SBUF: same DVE wall time as the
                    # sum row alone (free-size bound), and it releases the
                    # PSUM buffer for the next pair instance immediately
                    st["ssum"] = rec_pool.tile([DH + 1, SUPLEN], F32, name="ssum")
                    nc.vector.tensor_copy(st["ssum"], out2)
                    st["rsd"] = dram_pool.tile([1, SUPLEN], F32, name="rsd")
                    nc.sync.dma_start(st["rsd"], st["ssum"][DH : DH + 1, :])
                    st["spread"] = spread_pool.tile(
                        [128, SUPLEN // 128], F32, name="spread"
                    )
                    nc.sync.dma_start(
                        st["spread"],
                        st["rsd"][:, :].rearrange("a (p f) -> (a p) f", p=128),
                    )

                def s2():
                    nc.vector.reciprocal(st["spread"], st["spread"])
                    st["rsd2"] = dram_pool.tile([1, SUPLEN], F32, name="rsd2")
                    nc.sync.dma_start(
                        st["rsd2"][:, :].rearrange("a (p f) -> (a p) f", p=128),
                        st["spread"],
                    )
                    st["rb"] = rb_pool.tile([64, SUPLEN], F32, name="rb")
                    nc.sync.dma_start(st["rb"], st["rsd2"][:, :].partition_broadcast(64))

                def s3():
                    rb = st["rb"]
                    onum = st["ssum"][0:DH, :]
                    if h % 2 == 0:
                        seg = cc[mt][0:64, qsl]
                        nc.vector.tensor_mul(seg, onum, rb)
                        nc.vector.tensor_scalar_add(
                            seg, seg, scalar1=bv_sb[:, h : h + 1]
                        )
                    else:
                        segt = seg_pool.tile([64, SUPLEN], BF16, name="segt")
                        nc.vector.tensor_mul(segt, onum, rb)
                        nc.vector.tensor_scalar_add(
                            segt, segt, scalar1=bv_sb[:, h : h + 1]
                        )
                        # partition move 0-63 -> 64-127 via DMA
                        nc.sync.dma_start(cc[mt][64:128, qsl], segt)

                return [s1, s2, s3]

            # ---- phase 1b/2a: v projection with the attention ramp woven
            # in, so ScalarE saturates on exps while PE runs the v matmuls.
            xv_tiles = []
            for kc in range(KC):
                xt = xv_pool.tile([128, S], BF16, name="xvt")
                nc.sync.dma_start(xt, xvT[kc * 128 : (kc + 1) * 128, :])
                xv_tiles.append(xt)

            def emit_vproj(tks):
                for tk in tks:
                    ps = pacc.tile([128, CD], F32, name="ps_v", tag="pacc")
                    for kc in range(KC):
                        nc.tensor.matmul(
                            ps,
                            lhsT=xv_tiles[kc][:, tk * 128 : (tk + 1) * 128],
                            rhs=wv_sb[:, kc, :],
                            start=(kc == 0),
                            stop=(kc == KC - 1),
                        )
                    nc.vector.tensor_copy(
                        vv[:, :, tk, 0:DH],
                        ps.rearrange("p (h d) -> p h d", h=HPC),
                    )

            emit_vproj(range(4))

            ramp_abf = {}  # (h01, ck) -> tile
            for ck in range(RAMP):
                scs = emit_scores_pair(0, 0, ck)
                for h01 in range(2):
                    ramp_abf[(h01, ck)] = emit_exp_mul(0, ck, scs[h01])

            emit_vproj(range(4, TC))

            # ---- output projection tile emitter ----
            op_serial = [0]

            def outproj_tile(mo, nt, evac_engine=None):
                i = op_serial[0]
                op_serial[0] += 1
                pool, tag = (psc, "psc") if i % 2 == 0 else (pacc, "pacc")
                ps = pool.tile([128, NB], F32, name="ps_o", tag=tag)
                for kc in range(CD // 128):
                    nc.tensor.matmul(
                        ps,
                        lhsT=wo_sb[:, kc, mo * 128 : (mo + 1) * 128],
                        rhs=cc[kc][:, nt * NB : (nt + 1) * NB],
                        start=(kc == 0),
                        stop=(kc == CD // 128 - 1),
                    )
                ot = oev_pool.tile([128, NB], BF16, name="ot")
                if evac_engine == "vector" or (evac_engine is None and i % 2 == 0):
                    nc.vector.tensor_copy(ot, ps)
                else:
                    nc.scalar.copy(ot, ps)
                nc.sync.dma_start(
                    poutT[mo * 128 : (mo + 1) * 128, nt * NB : (nt + 1) * NB], ot
                )

            # ---- phase 2b: attention, per head-pair instance ----
            pendings = []  # queued epilogue stage lists

            def drain_pending(n):
                done = 0
                while pendings and done < n:
                    if not pendings[0]:
                        pendings.pop(0)
                        continue
                    pendings[0].pop(0)()
                    done += 1

            def drain_stage_each():
                # one stage from EVERY pending epilogue (releases each out2)
                for p in pendings:
                    if p:
                        p.pop(0)()
                while pendings and not pendings[0]:
                    pendings.pop(0)

            for sup in range(SUPS):
                for mt in range(2):
                    pair_start = sup == 0 and mt == 0
                    out2 = [
                        pacc.tile([DH + 1, SUPLEN], F32, name=f"out2_{h01}", tag="pacc")
                        for h01 in range(2)
                    ]
                    # run both s1 stages (out2 buffer release) right away so
                    # this pair's first attn@v never stalls on the PSUM pool
                    drain_stage_each()
                    for ck in range(TC):
                        ramped = pair_start and ck < RAMP
                        if not ramped:
                            scs = emit_scores_pair(sup, mt, ck)
                        for h01 in range(2):
                            h = 2 * mt + h01
                            if ramped:
                                abf = ramp_abf[(h01, ck)]
                            else:
                                abf = emit_exp_mul(sup, ck, scs[h01])
                            emit_attnv(h, ck, out2[h01], abf)
                        if ck in (2, 4, 6, 8):
                            drain_pending(1)
                    for h01 in range(2):
                        pendings.append(make_epilogue(sup, 2 * mt + h01, out2[h01]))

            # ---- phase 3: output projection tail ----
            emitted = 0
            for nt in range(NT):
                for mo in range(D // 128):
                    outproj_tile(mo, nt, evac_engine="scalar" if emitted < 10 else None)
                    emitted += 1
                    if emitted % 2 == 0:
                        drain_pending(1)
            drain_pending(100)

    nc.finalize()
    return nc


def make_in_maps(query, key, value, mask, chemical_bias, Wq, bq, Wk, bk, Wv, bv, Wo, S=S_FULL):
    """Host-side preprocessing: per-core input dicts (8 cores)."""
    f32 = np.float32

    def c(a, dt):
        return np.ascontiguousarray(a, dtype=dt)

    per_batch = []
    for b in range(B):
        xq = c(query[b].T, nbf16)
        xk = c(key[b].T, nbf16)
        xv = c(value[b].T, nbf16)
        bm = np.where(mask[b, 0] == 0, f32(0.0), np.exp(chemical_bias[b], dtype=f32))
        expbT_ = c(bm.T, nbf16)
        per_batch.append((xq, xk, xv, expbT_))

    def warr(wt, kc):
        # [kc*128, M] -> [128, kc*M]: per-partition-contiguous device layout
        m = wt.shape[1]
        return np.ascontiguousarray(
            wt.reshape(kc, 128, m).transpose(1, 0, 2).reshape(128, kc * m), nbf16
        )

    per_group = []
    for g in range(4):
        hsl = slice(g * CD, (g + 1) * CD)
        wqT_ = warr(np.asarray((Wq[hsl] / SCALE).T, np.float32), KC)
        wkT_ = warr(np.asarray(Wk[hsl].T, np.float32), KC)
        wvT_ = warr(np.asarray(Wv[hsl].T, np.float32), KC)
        woT_ = warr(np.asarray(Wo[:, hsl].T, np.float32), CD // 128)
        bqc_ = c((bq[hsl] / SCALE).reshape(2, 128).T, f32)
        bkc_ = c(bk[hsl].reshape(2, 128).T, f32)
        bvc_ = c(bv[hsl].reshape(HPC, 64).T, f32)
        per_group.append((wqT_, wkT_, wvT_, woT_, bqc_, bkc_, bvc_))

    in_maps = []
    for core in range(NCORES):
        b, g = divmod(core, 4)
        xq, xk, xv, expbT_ = per_batch[b]
        wqT_, wkT_, wvT_, woT_, bqc_, bkc_, bvc_ = per_group[g]
        in_maps.append(
            {
                "xqT": xq,
                "xkT": xk,
                "xvT": xv,
                "wqT": wqT_,
                "wkT": wkT_,
                "wvT": wvT_,
                "woT": woT_,
                "bqc": bqc_,
                "bkc": bkc_,
                "bvc": bvc_,
                "expbT": expbT_,
            }
        )
    return in_maps


def combine_outputs(results, bo):
    """Sum per-group transposed partials into the full [B, S, D] output."""
    out = np.empty((B, S_FULL, D), np.float32)
    for b in range(B):
        acc = results[4 * b]["poutT"].T.astype(np.float32)
        for g in range(1, 4):
            acc = acc + results[4 * b + g]["poutT"].T.astype(np.float32)
        out[b] = acc + bo.astype(np.float32)
    return out


_NC_CACHE = {}


def _get_module(S=S_FULL):
    key = (S,)
    if key not in _NC_CACHE:
        _NC_CACHE[key] = build_module(S)
    return _NC_CACHE[key]


def run_spmd(in_maps, S=S_FULL, **kwargs):
    from concourse.bass_utils import run_bass_kernel_spmd

    nc = _get_module(S)
    return run_bass_kernel_spmd(nc, in_maps, core_ids=list(range(NCORES)), **kwargs)


def kernel(query, key, value, mask, chemical_bias, Wq, bq, Wk, bk, Wv, bv, Wo, bo):
    in_maps = make_in_maps(
        query, key, value, mask, chemical_bias, Wq, bq, Wk, bk, Wv, bv, Wo
    )
    res = run_spmd(in_maps)
    return combine_outputs(res.results, bo)
